# revision 40
# baseline (speedup 1.0000x reference)
"""Trainium2 kernel for nn_CMSBlockLinear (block-sparse linear layer).

Strategy: the 50%-dense random block topology (16x16 blocks) is hostile to
the 128x128 PE array, so densify the weights host-side (pure index
plumbing, no FLOPs) and run a dense [8192,2048]x[2048,8192] matmul,
token-sharded 8 ways across NeuronCores.

All matmul operands are bf16: the PE streams bf16 at the same 1 elem/cell/
cycle as fp32r (measured 216ns per [128x128]x[128,512] MM = the hardware
floor), but W traffic halves to 32MB/core -- the fp32r version was
W-DMA-paced at ~147GB/s, exactly matching MM consumption. W DMAs fetch
ko-PAIRS ([128, 2x512] bf16 = 2KB per partition line) for DMA line
efficiency. fp8 cannot pass the 2e-2 gate (e4m3 rel err 3.8e-2) and
every hi/lo split scheme refunds the DoubleRow 2x exactly.

Per core: out[1024 tok, 8192 feat] = xT_shard.T @ W_dense
  - xT shard [128, 16, 1024] bf16 lives in SBUF; chunks are the
    stationary matmul operand.
  - W streamed in [128, 1024] bf16 tiles (2 ko-chunks, 256 KB DMAs),
    the moving operand.
  - psum [128 tok, 512 feat] fp32 accumulates over the 16 contraction
    chunks; output drains as bf16 (DVE cast -> scalar-queue DMA) in
    [tokens, features] layout; the host upcasts and concatenates the 8
    shards. The last feature-tile runs m-outer so its drain overlaps
    the final matmuls.
"""

import sys

sys.path.insert(0, "/opt/trn_rl_repo")

import numpy as np
import ml_dtypes

T, IN_F, OUT_F = 8192, 2048, 8192
NCORES = 8
TPC = T // NCORES  # 1024 tokens per core
KO = IN_F // 128  # 16 contraction chunks of 128
KP = KO // 2  # 8 ko-pairs per W DMA
NT = OUT_F // 512  # 16 feature tiles of 512
MT = TPC // 128  # 8 token tiles of 128
XBUFS = 17  # warm_r + xk0..xk15, all live at once

_cached_nc = None


def _build_program():
    global _cached_nc
    if _cached_nc is not None:
        return _cached_nc
    from concourse import bacc, mybir, tile

    F32, BF16 = mybir.dt.float32, mybir.dt.bfloat16

    nc = bacc.Bacc(None)
    xT = nc.declare_dram_parameter("xT", [128, KO, TPC], BF16, isOutput=False)
    W = nc.declare_dram_parameter("W", [NT, KP, 128, 1024], BF16, isOutput=False)
    out = nc.declare_dram_parameter("out", [TPC, OUT_F], BF16, isOutput=True)

    with tile.TileContext(nc) as tc:
        with tc.tile_pool(name="xt", bufs=1) as xpool, \
             tc.tile_pool(name="wt", bufs=10) as wpool, \
             tc.tile_pool(name="ot", bufs=12) as opool, \
             tc.tile_pool(name="ps", bufs=1, space="PSUM") as ps:
            # One tag per pool: teardown emits per-semaphore clears that scale
            # with total ring slots, so pools are kept as lean as possible.
            #
            # HAM pre-warm: the clock gate reaches 2.4GHz only after ~3.4us of
            # sustained PE activity, and the first DMA tiles land ~10us in
            # (runtime prologue ~6us + queue bring-up). 14 N=512 dummy matmuls
            # run ~8.4-14.7us; the overshoot past data-arrival is deliberate:
            # it lets the W queue bank a few tiles of headroom. A leaner
            # warmup (34x N=128 ending at ~10.2us) made the stream catch up
            # to the W-queue ramp at ~14.5us -> 2.8us stall -> HAM
            # re-throttle, net +4us. Measured best: this exact shape.
            # The tile framework requires every tile to be written before it
            # is read, so the warm tile cannot stay uninitialized; DVE
            # memset measured marginally better than gpsimd here. 9 N=512
            # matmuls give the HAM clock gate its 3.4us of continuous PE
            # busy and end right at data arrival (~12us).
            warm = xpool.tile([128, 1024], BF16, tag="x", bufs=XBUFS, name="warm_r")
            nc.vector.memset(warm[:, :512], 0.0)
            wps = ps.tile([128, 512], F32, tag="p", bufs=MT, name="warm_ps")
            for i in range(9):
                nc.tensor.matmul(
                    wps[:], warm[:, :128], warm[:, :512], start=True, stop=True
                )

            # x rides the gpsimd SWDGE queue: the scalar HW-DGE queue
            # serializes DIRECT2Ds at ~3.8us cadence early (~67GB/s), below
            # the 148GB/s the first n-tile consumes x at; SWDGE pre-generates
            # descriptors and measured 180-240GB/s sustained for x. xk0 is
            # NOT split into halves: SWDGE descgen cost is per descriptor
            # line (128 per tile regardless of width), so a split doubles
            # the descgen ahead of xk1.. and stalls the early stream.
            xts = []
            for ko in range(KO):
                xk = xpool.tile([128, TPC], BF16, tag="x", bufs=XBUFS, name=f"xk{ko}")
                nc.gpsimd.dma_start(out=xk[:], in_=xT[:, ko, :])
                xts.append(xk)

            def xap(ko, m):
                return xts[ko][:, m * 128 : (m + 1) * 128]

            def drain(n, m, psum):
                # bf16 output (host upcasts; +~2e-3 rel err, within tol).
                # All casts on DVE, all out dma_starts on the scalar HW
                # queue: measured best. ACT casts stall the psum-free
                # chain (+15us) and sync-queue out DMAs head-of-line
                # block the W stream (+1us).
                ot = opool.tile([128, 512], BF16, tag="o", name=f"o{n}_{m}")
                nc.vector.tensor_copy(ot[:], psum[:])
                nc.scalar.dma_start(
                    out=out[m * 128 : (m + 1) * 128, n * 512 : (n + 1) * 512],
                    in_=ot[:],
                )

            for n in range(NT - 1):
                psums = [
                    ps.tile([128, 512], F32, tag="p", bufs=MT, name=f"ps{n}_{m}")
                    for m in range(MT)
                ]
                for kp in range(KP):
                    if n == 0 and kp < 3:
                        # The sync HW-DGE queue serializes its first DMAs at
                        # one-DIRECT2D-per-completion cadence (~3.9us per
                        # 256KB tile). Fetching the first three ko-pairs as
                        # six 128KB half-tiles halves each link of that
                        # chain, so W supply covers ko 0-5 until the ring
                        # pipelines to full rate.
                        wa = wpool.tile([128, 512], BF16, tag="w", name=f"w0{kp}a")
                        nc.sync.dma_start(out=wa[:], in_=W[0, kp][:, 0:512])
                        wb = wpool.tile([128, 512], BF16, tag="w", name=f"w0{kp}b")
                        nc.sync.dma_start(out=wb[:], in_=W[0, kp][:, 512:1024])
                        halves = [wa[:], wb[:]]
                    else:
                        wt = wpool.tile(
                            [128, 1024], BF16, tag="w", name=f"w{n}_{kp}"
                        )
                        nc.sync.dma_start(out=wt[:], in_=W[n, kp])
                        halves = [wt[:, 0:512], wt[:, 512:1024]]
                    for half in range(2):
                        ko = 2 * kp + half
                        for m in range(MT):
                            nc.tensor.matmul(
                                psums[m][:],
                                xap(ko, m),
                                halves[half],
                                start=(ko == 0),
                                stop=(ko == KO - 1),
                            )
                for m in range(MT):
                    drain(n, m, psums[m])

            # Last n-tile runs m-OUTER: each psum finishes its 16-chunk
            # accumulation ~3.4us apart, so the DVE cast chain and out DMAs
            # overlap the remaining matmuls instead of serializing after the
            # final one (saves ~4us of drain tail). All 8 W tiles of this
            # n-tile are resident by now (the W stream runs ~30us ahead).
            n = NT - 1
            lwts = []
            for kp in range(KP):
                wt = wpool.tile([128, 1024], BF16, tag="w", name=f"w{n}_{kp}")
                nc.sync.dma_start(out=wt[:], in_=W[n, kp])
                lwts.append(wt)
            for m in range(MT):
                psum = ps.tile([128, 512], F32, tag="p", bufs=MT, name=f"ps{n}_{m}")
                for ko in range(KO):
                    nc.tensor.matmul(
                        psum[:],
                        xap(ko, m),
                        lwts[ko // 2][:, (ko % 2) * 512 : (ko % 2 + 1) * 512],
                        start=(ko == 0),
                        stop=(ko == KO - 1),
                    )
                if m < MT - 1:
                    drain(n, m, psum)
                else:
                    # The very last drain is split in half-width pairs: the
                    # first half's out-DMA transfer overlaps the second
                    # half's cast, and the final transfer (which gates the
                    # end-of-kernel completion wait) is half-sized.
                    for h in range(2):
                        ot = opool.tile(
                            [128, 256], BF16, tag="o", name=f"olast{h}"
                        )
                        nc.vector.tensor_copy(
                            ot[:], psum[:, h * 256 : (h + 1) * 256]
                        )
                        nc.scalar.dma_start(
                            out=out[
                                m * 128 : (m + 1) * 128,
                                n * 512 + h * 256 : n * 512 + (h + 1) * 256,
                            ],
                            in_=ot[:],
                        )
    nc.compile()
    _cached_nc = nc
    return nc


def _prep_inputs(x, values, bias, col_indices):
    x = np.ascontiguousarray(np.asarray(x), dtype=np.float32)
    values = np.ascontiguousarray(np.asarray(values), dtype=np.float32)
    bias = np.asarray(bias, dtype=np.float32)
    col_indices = np.asarray(col_indices, dtype=np.int32)

    R, K = col_indices.shape  # 512, 64
    C = IN_F // 16  # 128 column blocks

    # Scatter block values into the dense weight matrix Wd[k_in, n_out].
    Wb = np.zeros((C, R, 16, 16), np.float32)  # [c, r, i, o]
    r_idx = np.broadcast_to(np.arange(R, dtype=np.int64)[:, None], col_indices.shape)
    Wb[col_indices, r_idx] = values.transpose(0, 1, 3, 2)  # values[r,k,o,i] -> [i,o]
    Wd = Wb.transpose(0, 2, 1, 3).reshape(IN_F, OUT_F)
    Wd = Wd.astype(ml_dtypes.bfloat16)
    # [NT, KP, 128, 2, 512] -> per-partition line holds a contiguous
    # ko-pair (2KB in HBM) for DMA efficiency.
    W4 = np.ascontiguousarray(
        Wd.reshape(KP, 2, 128, NT, 512).transpose(3, 0, 2, 1, 4).reshape(NT, KP, 128, 1024)
    )

    in_maps = []
    for c in range(NCORES):
        xs = x[c * TPC : (c + 1) * TPC]  # [TPC, IN_F]
        xTc = np.ascontiguousarray(
            xs.T.reshape(KO, 128, TPC).transpose(1, 0, 2)
        ).astype(ml_dtypes.bfloat16)  # [128, KO, TPC]
        in_maps.append({"xT": xTc, "W": W4})
    return in_maps, bias


def _run(x, values, bias, col_indices, trace=False):
    from concourse.bass_utils import run_bass_kernel_spmd

    nc = _build_program()
    in_maps, bias_np = _prep_inputs(x, values, bias, col_indices)
    kwargs = {}
    if trace:
        import tempfile

        kwargs["tmpdir"] = tempfile.mkdtemp(prefix="bass_trace_")
    try:
        res = run_bass_kernel_spmd(
            nc, in_maps, list(range(NCORES)), trace=trace, **kwargs
        )
    except Exception:
        # Transient device wedges (NRT_EXEC_UNIT_UNRECOVERABLE) have been
        # observed to clear on retry.
        import time

        time.sleep(20)
        res = run_bass_kernel_spmd(
            nc, in_maps, list(range(NCORES)), trace=trace, **kwargs
        )
    out = np.concatenate(
        [res.results[c]["out"].astype(np.float32) for c in range(NCORES)], axis=0
    )
    if np.any(bias_np):
        out = out + bias_np[None, :]
    return out, res


def kernel(x, values, bias, col_indices):
    out, _ = _run(x, values, bias, col_indices)
    return out
